# revision 4
# baseline (speedup 1.0000x reference)
"""ConditionedPNA on trn2: device-resident message passing + dense PNA.

4-core mesh, one query batch per core (data-parallel over B per the hint).
Device-resident per batch: hid [NPAD,64], utab [NPAD,64] (= sigmoid(score)*
hidden gather table), score [NPAD,1], zc [NPAD,256] (table init pattern).
Per layer: host selects top-K nodes / top-ESEL edges (exact jax tie
semantics) on compact arrays, ships dst-sorted edges as uint16 [128,1280]
slot grids; device gathers u/relw rows, forms msg|msg^2, segment-reduces
into smq [NPAD,256] (sum|sq|max|min) via gather-modify-scatter rounds
(slot column c = dst-sorted positions {p*1280+c}: same-dst runs are
contiguous and short, so every column and every 8-column stride-160 link
has unique dst -> indirect DMA semantics exact), then does the dense PNA
update + score MLP on-device. Only the new score ships back (200KB/batch).
"""
import os
import sys

sys.path.insert(0, "/opt/trn_rl_repo")

import numpy as np

# ---------------- problem constants ----------------
B, N, E, D, R2, T, M, L = 4, 50000, 1600000, 64, 1000, 32, 10000, 3
K = 5000
ESEL = 160000
P = 128
NT = 392                      # node tiles
NPAD = NT * P                 # 50176
NCHUNK = 98                   # 512-node chunks
CW = 512
SLOTC = 1280                  # slot columns
NSLOT = P * SLOTC             # 163840
LINKS = 160                   # GMS links (8 cols each, stride LINKS)
MAXRUN = 1280                 # layout-safe max in-degree for column unique
LINKRUN = 160                 # max run so stride-160 link stays unique
MT = 79                       # text tiles
MPAD = MT * P                 # 10112
NCORES_USED = 4

_f32 = np.float32
_i32 = np.int32

_STATE = None


# ================= device program builders =================
def _prog_init(nc):
    import concourse.tile as tile
    import concourse.bass as bass
    from concourse import mybir
    from concourse.masks import make_identity
    dt = mybir.dt
    AF = mybir.ActivationFunctionType

    t_texts = nc.dram_tensor("texts", [MPAD, D], dt.float32, kind="ExternalInput")
    t_alloff = nc.dram_tensor("alloff", [P, MT], dt.int32, kind="ExternalInput")
    t_hemb = nc.dram_tensor("hemb", [P, D], dt.float32, kind="ExternalInput")
    t_hoff = nc.dram_tensor("hoff", [P, 1], dt.int32, kind="ExternalInput")
    t_hsc = nc.dram_tensor("hsc", [P, 1], dt.float32, kind="ExternalInput")
    t_sc0 = nc.dram_tensor("sc0", [NPAD, 1], dt.float32, kind="ExternalInput")

    t_hid = nc.dram_tensor("hid", [NPAD, D], dt.float32, kind="ExternalOutput")
    t_utab = nc.dram_tensor("utab", [NPAD, D], dt.float32, kind="ExternalOutput")
    t_score = nc.dram_tensor("score", [NPAD, 1], dt.float32, kind="ExternalOutput")
    t_zc = nc.dram_tensor("zc", [NPAD, 256], dt.float32, kind="ExternalOutput")

    with tile.TileContext(nc) as tc:
        with (tc.tile_pool(name="big", bufs=1) as big,
              tc.tile_pool(name="wk", bufs=3) as wk,
              tc.tile_pool(name="ps", bufs=4, space="PSUM") as ps):
            ident = big.tile([P, P], dt.float32)
            make_identity(nc, ident[:])

            # zc pattern + zero hidden
            zpat = big.tile([P, 256], dt.float32)
            nc.vector.memset(zpat[:, 0:128], 0.0)
            nc.vector.memset(zpat[:, 128:192], -1e38)
            nc.vector.memset(zpat[:, 192:256], 1e38)
            for t in range(NT):
                nc.sync.dma_start(t_zc[t * P:(t + 1) * P, :], zpat[:])
                nc.sync.dma_start(t_hid[t * P:(t + 1) * P, :], zpat[:, 0:D])

            # scatter text embeddings (dedup'd, last-wins on host), then head
            txt = big.tile([P, MT * D], dt.float32)
            nc.sync.dma_start(
                txt[:].rearrange("p (c d) -> p c d", c=MT),
                t_texts[:].rearrange("(c p) d -> p c d", p=P))
            aoff = big.tile([P, MT], dt.int32)
            nc.sync.dma_start(aoff[:], t_alloff[:])
            for c in range(MT):
                nc.gpsimd.indirect_dma_start(
                    out=t_hid[:], out_offset=bass.IndirectOffsetOnAxis(
                        ap=aoff[:, c:c + 1], axis=0),
                    in_=txt[:, c * D:(c + 1) * D], in_offset=None,
                    bounds_check=NPAD - 1, oob_is_err=False)
            hemb = wk.tile([P, D], dt.float32, tag="hemb")
            nc.sync.dma_start(hemb[:], t_hemb[:])
            hoff = big.tile([P, 1], dt.int32)
            nc.sync.dma_start(hoff[:], t_hoff[:])
            nc.gpsimd.indirect_dma_start(
                out=t_hid[:], out_offset=bass.IndirectOffsetOnAxis(
                    ap=hoff[:, 0:1], axis=0),
                in_=hemb[:], in_offset=None,
                bounds_check=NPAD - 1, oob_is_err=False)

            # score0 (host grid) + head score scatter
            nc.sync.dma_start(t_score[:], t_sc0[:])
            hsc = wk.tile([P, 1], dt.float32, tag="hsc")
            nc.sync.dma_start(hsc[:], t_hsc[:])
            nc.gpsimd.indirect_dma_start(
                out=t_score[:], out_offset=bass.IndirectOffsetOnAxis(
                    ap=hoff[:, 0:1], axis=0),
                in_=hsc[:], in_offset=None,
                bounds_check=NPAD - 1, oob_is_err=False)

            # gate tile-major from score (chunk-major load + transposes)
            sC = big.tile([NCHUNK, CW], dt.float32)
            nc.sync.dma_start(
                sC[:], t_score[:].rearrange("(k w) o -> k (w o)", k=NCHUNK))
            gC = big.tile([NCHUNK, CW], dt.float32)
            nc.scalar.activation(gC[:], sC[:], AF.Sigmoid)
            gM = big.tile([P, NT], dt.float32)
            for q in range(4):
                pt = ps.tile([P, NCHUNK], dt.float32, tag="tg")
                nc.tensor.transpose(pt[:], gC[:, q * P:(q + 1) * P],
                                    ident[0:NCHUNK, 0:NCHUNK])
                nc.vector.tensor_copy(gM[:, q::4], pt[:])

            # utab = gate * hidden (node-major tiles)
            for t in range(NT):
                h0 = wk.tile([P, D], dt.float32, tag="h0")
                nc.sync.dma_start(h0[:], t_hid[t * P:(t + 1) * P, :])
                u0 = wk.tile([P, D], dt.float32, tag="u0")
                nc.vector.tensor_scalar_mul(u0[:], h0[:], gM[:, t:t + 1])
                nc.sync.dma_start(t_utab[t * P:(t + 1) * P, :], u0[:])


def _prog_layer(nc):
    import concourse.tile as tile
    import concourse.bass as bass
    from concourse import mybir
    from concourse.masks import make_identity
    dt = mybir.dt
    AOp = mybir.AluOpType
    AF = mybir.ActivationFunctionType

    t_sv = nc.dram_tensor("sv", [P, SLOTC], dt.uint16, kind="ExternalInput")
    t_et = nc.dram_tensor("et", [P, SLOTC], dt.uint16, kind="ExternalInput")
    t_dv = nc.dram_tensor("dv", [P, SLOTC], dt.uint16, kind="ExternalInput")
    t_degt = nc.dram_tensor("degt", [P, NT], dt.uint8, kind="ExternalInput")
    t_degc = nc.dram_tensor("degc", [NCHUNK, CW], dt.uint8, kind="ExternalInput")
    t_wrel = nc.dram_tensor("wrel", [R2, D], dt.float32, kind="ExternalInput")
    t_wpna = nc.dram_tensor("wpna", [12 * D + 1, D], dt.float32, kind="ExternalInput")
    t_lh = nc.dram_tensor("lh", [D, D], dt.float32, kind="ExternalInput")
    t_w1 = nc.dram_tensor("w1", [D, 2 * D], dt.float32, kind="ExternalInput")
    t_w2 = nc.dram_tensor("w2", [2 * D, 1], dt.float32, kind="ExternalInput")
    t_b1 = nc.dram_tensor("b1", [2 * D, 1], dt.float32, kind="ExternalInput")
    t_hb = nc.dram_tensor("hb", [D, 1], dt.float32, kind="ExternalInput")
    t_b2 = nc.dram_tensor("b2", [1, 1], dt.float32, kind="ExternalInput")
    t_utab = nc.dram_tensor("utab", [NPAD, D], dt.float32, kind="ExternalInput")
    t_hid = nc.dram_tensor("hid", [NPAD, D], dt.float32, kind="ExternalInput")
    t_score = nc.dram_tensor("score", [NPAD, 1], dt.float32, kind="ExternalInput")
    t_zc = nc.dram_tensor("zc", [NPAD, 256], dt.float32, kind="ExternalInput")

    t_hid_o = nc.dram_tensor("hid_o", [NPAD, D], dt.float32, kind="ExternalOutput")
    t_utab_o = nc.dram_tensor("utab_o", [NPAD, D], dt.float32, kind="ExternalOutput")
    t_score_o = nc.dram_tensor("score_o", [NPAD, 1], dt.float32, kind="ExternalOutput")

    t_smq = nc.dram_tensor("smq", [NPAD, 256], dt.float32, kind="Internal")
    t_sst = nc.dram_tensor("sst", [NPAD, 1], dt.float32, kind="Internal")

    with tile.TileContext(nc) as tc:
        with (tc.tile_pool(name="big", bufs=1) as big,
              tc.tile_pool(name="wk", bufs=4) as wk,
              tc.tile_pool(name="gat", bufs=8) as gat,
              tc.tile_pool(name="psT", bufs=2, space="PSUM") as psT,
              tc.tile_pool(name="psG", bufs=1, space="PSUM") as psG,
              tc.tile_pool(name="psS", bufs=1, space="PSUM") as psS):
            ident = big.tile([P, P], dt.float32)
            make_identity(nc, ident[:])
            ones128 = big.tile([1, P], dt.float32)
            nc.vector.memset(ones128[:], 1.0)

            # reset smq
            nc.sync.dma_start(t_smq[:], t_zc[:])

            # slot grids -> int32
            svi = big.tile([P, SLOTC], dt.int32)
            eti = big.tile([P, SLOTC], dt.int32)
            dvi = big.tile([P, SLOTC], dt.int32)
            for (src, dst) in ((t_sv, svi), (t_et, eti), (t_dv, dvi)):
                u16 = wk.tile([P, SLOTC], dt.uint16, tag="u16")
                nc.sync.dma_start(u16[:], src[:])
                nc.vector.tensor_copy(dst[:], u16[:])

            # ---- scatter phase: LINKS links x 8 columns (stride LINKS)
            for ln in range(LINKS):
                cols = [ln + LINKS * s for s in range(8)]
                pairs = []
                for c in cols:
                    gu = gat.tile([P, D], dt.float32, tag="gu")
                    nc.gpsimd.indirect_dma_start(
                        out=gu[:], out_offset=None, in_=t_utab[:],
                        in_offset=bass.IndirectOffsetOnAxis(
                            ap=svi[:, c:c + 1], axis=0),
                        bounds_check=NPAD - 1, oob_is_err=False)
                    gw = gat.tile([P, D], dt.float32, tag="gw")
                    nc.gpsimd.indirect_dma_start(
                        out=gw[:], out_offset=None, in_=t_wrel[:],
                        in_offset=bass.IndirectOffsetOnAxis(
                            ap=eti[:, c:c + 1], axis=0),
                        bounds_check=R2 - 1, oob_is_err=False)
                    pair = gat.tile([P, 2 * D], dt.float32, tag="pair")
                    nc.vector.tensor_mul(pair[:, 0:D], gu[:], gw[:])
                    nc.scalar.square(pair[:, D:2 * D], pair[:, 0:D])
                    pairs.append(pair)
                curs = []
                for c in cols:
                    cur = gat.tile([P, 256], dt.float32, tag="cur")
                    nc.gpsimd.indirect_dma_start(
                        out=cur[:], out_offset=None, in_=t_smq[:],
                        in_offset=bass.IndirectOffsetOnAxis(
                            ap=dvi[:, c:c + 1], axis=0),
                        bounds_check=NPAD - 1, oob_is_err=False)
                    curs.append(cur)
                for i, c in enumerate(cols):
                    pair, cur = pairs[i], curs[i]
                    upd = gat.tile([P, 256], dt.float32, tag="upd")
                    nc.vector.tensor_add(upd[:, 0:128], cur[:, 0:128], pair[:])
                    nc.vector.tensor_tensor(upd[:, 128:192], cur[:, 128:192],
                                            pair[:, 0:D], op=AOp.max)
                    nc.vector.tensor_tensor(upd[:, 192:256], cur[:, 192:256],
                                            pair[:, 0:D], op=AOp.min)
                    nc.gpsimd.indirect_dma_start(
                        out=t_smq[:], out_offset=bass.IndirectOffsetOnAxis(
                            ap=dvi[:, c:c + 1], axis=0),
                        in_=upd[:], in_offset=None,
                        bounds_check=NPAD - 1, oob_is_err=False)

            # ---- per-node scalars (tile-major)
            degt8 = big.tile([P, NT], dt.uint8)
            nc.sync.dma_start(degt8[:], t_degt[:])
            degf = big.tile([P, NT], dt.float32)
            nc.vector.tensor_copy(degf[:], degt8[:])
            hasf = big.tile([P, NT], dt.float32)
            nc.vector.tensor_scalar(hasf[:], degf[:], 0.0, None, op0=AOp.is_gt)
            hasu8 = big.tile([P, NT], dt.uint8)
            nc.vector.tensor_scalar(hasu8[:], degf[:], 0.0, None, op0=AOp.is_gt)
            dgc = wk.tile([P, NT], dt.float32, tag="dgc")
            nc.vector.tensor_scalar_max(dgc[:], degf[:], 1.0)
            recipd = big.tile([P, NT], dt.float32)
            nc.vector.reciprocal(recipd[:], dgc[:])
            ampM = big.tile([P, NT], dt.float32)
            nc.scalar.activation(ampM[:], degf[:], AF.Ln, bias=1.0)
            logc = wk.tile([P, NT], dt.float32, tag="logc")
            nc.vector.tensor_scalar_max(logc[:], ampM[:], 1e-6)
            attM = big.tile([P, NT], dt.float32)
            nc.vector.reciprocal(attM[:], logc[:])
            nc.vector.tensor_mul(attM[:], attM[:], hasf[:])

            # ---- weights
            wpna = big.tile([D, 12 * D], dt.float32)
            nc.sync.dma_start(
                wpna[:].rearrange("k (b d) -> k b d", b=12),
                t_wpna[0:12 * D, :].rearrange("(b k) d -> k b d", k=D))
            pnarow = big.tile([1, D], dt.float32)
            nc.sync.dma_start(pnarow[:], t_wpna[12 * D:12 * D + 1, :])
            pb_ps = psT.tile([P, P], dt.float32, tag="t")
            nc.tensor.matmul(pb_ps[:, 0:D], ones128[:], pnarow[:], start=True, stop=True)
            pnabB = big.tile([P, D], dt.float32)
            nc.vector.tensor_copy(pnabB[:], pb_ps[:, 0:D])
            lh = big.tile([D, D], dt.float32)
            nc.sync.dma_start(lh[:], t_lh[:])
            w1 = big.tile([D, 2 * D], dt.float32)
            nc.sync.dma_start(w1[:], t_w1[:])
            w2 = big.tile([2 * D, 1], dt.float32)
            nc.sync.dma_start(w2[:], t_w2[:])
            b1 = big.tile([2 * D, 1], dt.float32)
            nc.sync.dma_start(b1[:], t_b1[:])
            hb = big.tile([D, 1], dt.float32)
            nc.sync.dma_start(hb[:], t_hb[:])
            b2 = big.tile([1, 1], dt.float32)
            nc.sync.dma_start(b2[:], t_b2[:])

            # ---- PNA + score_fn per 512-node chunk
            for k in range(NCHUNK):
                a_mean = wk.tile([D, CW], dt.float32, tag="a_mean")
                a_std = wk.tile([D, CW], dt.float32, tag="a_std")
                a_mx = wk.tile([D, CW], dt.float32, tag="a_mx")
                a_mn = wk.tile([D, CW], dt.float32, tag="a_mn")
                hTc = wk.tile([D, CW], dt.float32, tag="hTc")
                for t4 in range(4):
                    t = 4 * k + t4
                    smt = wk.tile([P, 256], dt.float32, tag="smt")
                    nc.sync.dma_start(smt[:], t_smq[t * P:(t + 1) * P, :])
                    rd = recipd[:, t:t + 1]
                    hs = hasf[:, t:t + 1]
                    agg = wk.tile([P, 256], dt.float32, tag="agg")
                    nc.vector.tensor_scalar_mul(agg[:, 0:D], smt[:, 0:D], rd)
                    ex2 = wk.tile([P, D], dt.float32, tag="ex2")
                    nc.vector.tensor_scalar_mul(ex2[:], smt[:, D:2 * D], rd)
                    msq = wk.tile([P, D], dt.float32, tag="msq")
                    nc.scalar.square(msq[:], agg[:, 0:D])
                    nc.vector.tensor_sub(ex2[:], ex2[:], msq[:])
                    nc.vector.tensor_scalar(ex2[:], ex2[:], 0.0, 1e-6,
                                            op0=AOp.max, op1=AOp.add)
                    nc.scalar.sqrt(ex2[:], ex2[:])
                    nc.vector.tensor_scalar_mul(agg[:, D:2 * D], ex2[:], hs)
                    nc.vector.tensor_scalar_mul(agg[:, 2 * D:3 * D],
                                                smt[:, 2 * D:3 * D], hs)
                    nc.vector.tensor_scalar_mul(agg[:, 3 * D:4 * D],
                                                smt[:, 3 * D:4 * D], hs)
                    pt = psT.tile([P, P], dt.float32, tag="t")
                    nc.tensor.transpose(pt[:], agg[:, 0:128], ident[:])
                    sl = slice(t4 * P, (t4 + 1) * P)
                    nc.vector.tensor_copy(a_mean[:, sl], pt[0:D, :])
                    nc.vector.tensor_copy(a_std[:, sl], pt[D:2 * D, :])
                    pt2 = psT.tile([P, P], dt.float32, tag="t")
                    nc.tensor.transpose(pt2[:], agg[:, 128:256], ident[:])
                    nc.vector.tensor_copy(a_mx[:, sl], pt2[0:D, :])
                    nc.vector.tensor_copy(a_mn[:, sl], pt2[D:2 * D, :])

                aggs = (a_mean, a_mx, a_mn, a_std)
                for t4 in range(4):
                    t = 4 * k + t4
                    sl = slice(t4 * P, (t4 + 1) * P)
                    pgs = []
                    for s in range(3):
                        pg = psG.tile([P, D], dt.float32, tag=f"pg{s}")
                        for a in range(4):
                            bidx = a * 3 + s
                            nc.tensor.matmul(
                                pg[:], aggs[a][:, sl],
                                wpna[:, bidx * D:(bidx + 1) * D],
                                start=(a == 0), stop=(a == 3))
                        pgs.append(pg)
                    hload = wk.tile([P, D], dt.float32, tag="hload")
                    nc.sync.dma_start(hload[:], t_hid[t * P:(t + 1) * P, :])
                    o2 = wk.tile([P, D], dt.float32, tag="o2")
                    nc.vector.tensor_scalar_mul(o2[:], pgs[1][:], ampM[:, t:t + 1])
                    o3 = wk.tile([P, D], dt.float32, tag="o3")
                    nc.vector.tensor_scalar_mul(o3[:], pgs[2][:], attM[:, t:t + 1])
                    out = wk.tile([P, D], dt.float32, tag="outc")
                    nc.vector.tensor_add(out[:], pgs[0][:], o2[:])
                    nc.vector.tensor_add(out[:], out[:], o3[:])
                    nc.vector.tensor_add(out[:], out[:], pnabB[:])
                    hnew = wk.tile([P, D], dt.float32, tag="hnew")
                    nc.vector.tensor_add(hnew[:], hload[:], out[:])
                    hsel = wk.tile([P, D], dt.float32, tag="hsel")
                    nc.vector.select(
                        hsel[:], hasu8[:, t:t + 1].to_broadcast([P, D]),
                        hnew[:], hload[:])
                    nc.sync.dma_start(t_hid_o[t * P:(t + 1) * P, :], hsel[:])
                    pth = psT.tile([P, P], dt.float32, tag="t")
                    nc.tensor.transpose(pth[0:D, :], hsel[:], ident[:])
                    nc.vector.tensor_copy(hTc[:, sl], pth[0:D, :])

                # score_fn on this chunk
                ph = psS.tile([D, CW], dt.float32, tag="ph")
                nc.tensor.matmul(ph[:], lh[:], hTc[:], start=True, stop=True)
                xt = wk.tile([D, CW], dt.float32, tag="xt")
                nc.vector.tensor_scalar_add(xt[:], ph[:], hb[:, 0:1])
                nc.vector.tensor_mul(xt[:], xt[:], hTc[:])
                p1 = psS.tile([2 * D, CW], dt.float32, tag="p1")
                nc.tensor.matmul(p1[:], w1[:], xt[:], start=True, stop=True)
                h1 = wk.tile([2 * D, CW], dt.float32, tag="h1")
                nc.scalar.activation(h1[:], p1[:], AF.Relu, bias=b1[:, 0:1])
                p2s = psS.tile([1, CW], dt.float32, tag="p2s")
                nc.tensor.matmul(p2s[:], w2[:], h1[:], start=True, stop=True)
                srow = wk.tile([1, CW], dt.float32, tag="srow")
                nc.vector.tensor_scalar_add(srow[:], p2s[:], b2[0:1, 0:1])
                nc.sync.dma_start(
                    t_sst[k * CW:(k + 1) * CW, :].rearrange("n o -> o n"),
                    srow[:])

            # ---- score select + gate + utab
            sC = big.tile([NCHUNK, CW], dt.float32)
            nc.sync.dma_start(
                sC[:], t_sst[:].rearrange("(k w) o -> k (w o)", k=NCHUNK))
            soldC = big.tile([NCHUNK, CW], dt.float32)
            nc.sync.dma_start(
                soldC[:], t_score[:].rearrange("(k w) o -> k (w o)", k=NCHUNK))
            dc8 = wk.tile([NCHUNK, CW], dt.uint8, tag="dc8")
            nc.sync.dma_start(dc8[:], t_degc[:])
            dcf = wk.tile([NCHUNK, CW], dt.float32, tag="dcf")
            nc.vector.tensor_copy(dcf[:], dc8[:])
            hasC = big.tile([NCHUNK, CW], dt.uint8)
            nc.vector.tensor_scalar(hasC[:], dcf[:], 0.0, None, op0=AOp.is_gt)
            snewC = big.tile([NCHUNK, CW], dt.float32)
            nc.vector.select(snewC[:], hasC[:], sC[:], soldC[:])
            nc.sync.dma_start(
                t_score_o[:].rearrange("(k w) o -> k (w o)", k=NCHUNK), snewC[:])
            gC = big.tile([NCHUNK, CW], dt.float32)
            nc.scalar.activation(gC[:], snewC[:], AF.Sigmoid)
            gM = big.tile([P, NT], dt.float32)
            for q in range(4):
                ptg = psT.tile([P, P], dt.float32, tag="t")
                nc.tensor.transpose(ptg[:, 0:NCHUNK], gC[:, q * P:(q + 1) * P],
                                    ident[0:NCHUNK, 0:NCHUNK])
                nc.vector.tensor_copy(gM[:, q::4], ptg[:, 0:NCHUNK])
            for t in range(NT):
                hload = wk.tile([P, D], dt.float32, tag="hl2")
                nc.sync.dma_start(hload[:], t_hid_o[t * P:(t + 1) * P, :])
                u0 = wk.tile([P, D], dt.float32, tag="u0")
                nc.vector.tensor_scalar_mul(u0[:], hload[:], gM[:, t:t + 1])
                nc.sync.dma_start(t_utab_o[t * P:(t + 1) * P, :], u0[:])


# ================= runner plumbing =================
def _make_runner(build_fn, donate_in=(), replicated=()):
    import concourse.bacc as bacc
    from concourse.bass2jax import (
        install_neuronx_cc_hook, _bass_exec_p, partition_id_tensor,
    )
    import jax
    from jax.sharding import Mesh, PartitionSpec
    from jax.experimental.shard_map import shard_map
    from concourse import mybir as mb

    nc = bacc.Bacc(target_bir_lowering=False)
    build_fn(nc)
    nc.finalize()
    install_neuronx_cc_hook()

    partition_name = nc.partition_id_tensor.name if nc.partition_id_tensor else None
    in_names, out_names, out_avals = [], [], []
    for alloc in nc.m.functions[0].allocations:
        if not isinstance(alloc, mb.MemoryLocationSet):
            continue
        name = alloc.memorylocations[0].name
        if alloc.kind == "ExternalInput":
            if name != partition_name:
                in_names.append(name)
        elif alloc.kind == "ExternalOutput":
            out_names.append(name)
            out_avals.append(jax.core.ShapedArray(
                tuple(alloc.tensor_shape), mb.dt.np(alloc.dtype)))
    all_names = list(in_names) + list(out_names)
    if partition_name is not None:
        all_names.append(partition_name)

    def _body(*args):
        operands = list(args)
        if partition_name is not None:
            operands.append(partition_id_tensor())
        return tuple(_bass_exec_p.bind(
            *operands, out_avals=tuple(out_avals), in_names=tuple(all_names),
            out_names=tuple(out_names), lowering_input_output_aliases=(),
            sim_require_finite=False, sim_require_nnan=False, nc=nc))

    devices = jax.devices()[:NCORES_USED]
    mesh = Mesh(np.asarray(devices), ("core",))
    nin = len(in_names)
    nout = len(out_avals)
    donate = tuple(in_names.index(n) for n in donate_in) + tuple(
        range(nin, nin + nout))
    in_specs = tuple(
        PartitionSpec() if n in replicated else PartitionSpec("core")
        for n in in_names) + (PartitionSpec("core"),) * nout
    sharded = jax.jit(shard_map(_body, mesh=mesh,
                                in_specs=in_specs,
                                out_specs=(PartitionSpec("core"),) * nout,
                                check_rep=False),
                      donate_argnums=donate, keep_unused=True)
    return {"in_names": in_names, "out_names": out_names,
            "out_avals": out_avals, "sharded": sharded, "mesh": mesh}


def _dev_zeros_factory():
    import jax
    import jax.numpy as jnp
    from jax.sharding import NamedSharding, PartitionSpec
    cache = {}

    def make(info):
        key = id(info)
        if key not in cache:
            mesh = info["mesh"]
            sh = NamedSharding(mesh, PartitionSpec("core"))
            fns = []
            for av in info["out_avals"]:
                shape = (NCORES_USED * av.shape[0],) + tuple(av.shape[1:])
                fns.append(jax.jit(
                    lambda shape=shape, dt=av.dtype: jnp.zeros(shape, dt),
                    out_shardings=sh))
            cache[key] = fns
        return [f() for f in cache[key]]
    return make


_dev_zeros = None


def _get_programs():
    global _STATE, _dev_zeros
    if _STATE is None:
        init_info = _make_runner(_prog_init,
                                 replicated=("texts", "alloff"))
        layer_info = _make_runner(_prog_layer,
                                  donate_in=("utab", "hid", "score"),
                                  replicated=("wrel", "wpna", "lh", "w1",
                                              "w2", "b1", "b2"))
        _dev_zeros = _dev_zeros_factory()
        _STATE = (init_info, layer_info)
    return _STATE


def _call(info, arrays_by_name, out_bufs):
    args = [arrays_by_name[n] for n in info["in_names"]]
    outs = info["sharded"](*args, *out_bufs)
    return dict(zip(info["out_names"], outs))


# ================= host-side exact selection =================
def _sigmoid(x):
    x = x.astype(_f32)
    out = np.empty_like(x)
    pos = x >= 0
    out[pos] = (1.0 / (1.0 + np.exp(-x[pos]))).astype(_f32)
    ex = np.exp(x[~pos]).astype(_f32)
    out[~pos] = ex / (1.0 + ex)
    return out.astype(_f32)


def _score_fn(hidden, rel, linear_w, linear_b, mlp_w1, mlp_b1, mlp_w2, mlp_b2):
    heur = hidden @ linear_w[:D] + rel @ linear_w[D:] + linear_b
    x = hidden * heur
    h1 = np.maximum(x @ mlp_w1 + mlp_b1, 0.0)
    return (h1 @ mlp_w2 + mlp_b2).astype(_f32)[:, 0]


def _topk_sel(score, k):
    """Exact lax.top_k set: k largest, ties -> lowest index."""
    v = np.partition(score, N - k)[N - k]
    gt = score > v
    g = int(gt.sum())
    sel = gt
    if g < k:
        eq_idx = np.nonzero(score == v)[0][:k - g]
        sel = gt.copy()
        sel[eq_idx] = True
    return sel


def _select_edges(score, sel, csr):
    """Exact top-ESEL edge set (ties by lowest original edge index).
    Returns (src, dst, typ) arrays of the selected valid edges."""
    indptr, sdst, styp, sorder, sel_src = csr
    sel_nodes = np.nonzero(sel)[0]
    st = indptr[sel_nodes]
    en = indptr[sel_nodes + 1]
    lens = en - st
    tot = int(lens.sum())
    if tot == 0:
        z = np.zeros(0, _i32)
        return z, z, z
    base = np.repeat(st - np.concatenate([[0], np.cumsum(lens)[:-1]]), lens)
    cand = base + np.arange(tot, dtype=np.int64)
    esc = score[sdst[cand]]
    if tot <= ESEL:
        keep = np.ones(tot, bool)
    else:
        v = np.partition(esc, tot - ESEL)[tot - ESEL]
        keep = esc > v
        g = int(keep.sum())
        need = ESEL - g
        if need > 0:
            eqm = esc == v
            eq_pos = np.nonzero(eqm)[0]
            eor = sorder[cand[eq_pos]]
            if len(eq_pos) > need:
                kth = np.partition(eor, need - 1)[need - 1]
                sub = eor <= kth
                keep[eq_pos[sub]] = True
            else:
                keep[eq_pos] = True
    candk = cand[keep]
    src = np.repeat(sel_nodes, lens)[keep].astype(_i32)
    dst = sdst[candk]
    typ = styp[candk]
    return src, dst, typ


# ================= main =================
def kernel(h_index, r_index, t_index, all_index, edge_src, edge_dst, edge_type,
           hidden_states, score_text_embs, rel_table, linear_w, linear_b,
           mlp_w1, mlp_b1, mlp_w2, mlp_b2, relw, pna_w, pna_b):
    h_index = np.asarray(h_index)
    r_index = np.asarray(r_index)
    t_index = np.asarray(t_index)
    all_index = np.asarray(all_index)
    edge_src = np.asarray(edge_src).astype(np.int64)
    edge_dst = np.asarray(edge_dst).astype(np.int64)
    edge_type = np.asarray(edge_type).astype(np.int64)
    hidden_states = np.asarray(hidden_states, _f32)
    score_text_embs = np.asarray(score_text_embs, _f32)
    rel_table = np.asarray(rel_table, _f32)
    linear_w = np.asarray(linear_w, _f32)
    linear_b = np.asarray(linear_b, _f32)
    mlp_w1 = np.asarray(mlp_w1, _f32)
    mlp_b1 = np.asarray(mlp_b1, _f32)
    mlp_w2 = np.asarray(mlp_w2, _f32)
    mlp_b2 = np.asarray(mlp_b2, _f32)
    relw = np.asarray(relw, _f32)
    pna_w = np.asarray(pna_w, _f32)
    pna_b = np.asarray(pna_b, _f32)

    deg_out_full = np.bincount(edge_src, minlength=N).astype(_f32)
    dmean = float(np.mean(np.log(deg_out_full + 1.0, dtype=_f32), dtype=_f32))

    # CSR by src (once)
    sorder = np.argsort(edge_src, kind="stable").astype(np.int64)
    ssrc = edge_src[sorder].astype(_i32)
    sdst = edge_dst[sorder].astype(_i32)
    styp = edge_type[sorder].astype(_i32)
    indptr = np.zeros(N + 1, np.int64)
    np.cumsum(np.bincount(ssrc, minlength=N), out=indptr[1:])
    csr = (indptr, sdst, styp, sorder.astype(np.int64), ssrc)

    sf = lambda h, r: _score_fn(h, r, linear_w, linear_b,
                                mlp_w1, mlp_b1, mlp_w2, mlp_b2)

    init_info, layer_info = _get_programs()

    # ---- init call inputs
    # dedup all_index keeping LAST occurrence (matches .at[].set order)
    rev_uniq, rev_first = np.unique(all_index[::-1], return_index=True)
    last_pos = len(all_index) - 1 - rev_first       # original positions
    mu = len(rev_uniq)
    texts = np.zeros((MPAD, D), _f32)
    alloff = np.full((P, MT), 1 << 20, _i32)
    # text slot (p, c) <-> texts row c*128+p
    tr = np.zeros(MPAD, np.int64)
    tr[:mu] = rev_uniq
    texts[:mu] = score_text_embs[last_pos]
    alloff.T.flat[:mu] = rev_uniq.astype(_i32)      # row-major over (c, p)

    base_sc = np.empty(B, _f32)
    head_sc = np.empty(B, _f32)
    for b in range(B):
        rel = rel_table[r_index[b]]
        base_sc[b] = sf(np.zeros((1, D), _f32), rel)[0]
        head_sc[b] = sf(hidden_states[b][None], rel)[0]

    def catb(fn):
        return np.concatenate([fn(b) for b in range(NCORES_USED)], axis=0)

    init_in = {
        "texts": texts,
        "alloff": alloff,
        "hemb": catb(lambda b: np.broadcast_to(hidden_states[b], (P, D)).copy()),
        "hoff": catb(lambda b: _mkhoff(int(h_index[b]))),
        "hsc": catb(lambda b: np.full((P, 1), head_sc[b], _f32)),
        "sc0": catb(lambda b: np.full((NPAD, 1), base_sc[b], _f32)),
    }
    prof = bool(os.environ.get("PNA_PROF"))
    import time as _time
    _t0 = _time.time()
    init_bufs = _dev_zeros(init_info)
    st = _call(init_info, init_in, init_bufs)
    hid_r, utab_r, score_r, zc_r = (st["hid"], st["utab"], st["score"], st["zc"])

    # host copy of score for selection
    score_h = np.asarray(score_r).reshape(NCORES_USED, NPAD)[:, :N].copy()
    if prof:
        print(f"[prof] init call+fetch: {_time.time()-_t0:.3f}s", flush=True)

    # per-layer static weights
    lh_in = linear_w[:D]
    w1_in = mlp_w1
    w2_in = mlp_w2
    b1_in = mlp_b1[:, None].astype(_f32)
    hb_in = catb(lambda b: (rel_table[r_index[b]] @ linear_w[D:]
                            + linear_b)[:, None].astype(_f32))
    b2_in = mlp_b2[:, None].astype(_f32)

    fallback = np.zeros(B, bool)
    for l in range(L):
        _t1 = _time.time()
        svs, ets, dvs, degts, degcs = [], [], [], [], []
        for b in range(B):
            sel = _topk_sel(score_h[b], K)
            src, dst, typ = _select_edges(score_h[b], sel, csr)
            cnt = np.bincount(dst, minlength=NPAD)
            if cnt.max() > LINKRUN:
                fallback[b] = True
                src = src[:0]; dst = dst[:0]; typ = typ[:0]
                cnt[:] = 0
            order = np.argsort(dst)
            n = len(order)
            svg = np.zeros((P, SLOTC), np.uint16)
            etg = np.zeros((P, SLOTC), np.uint16)
            dvg = np.full((P, SLOTC), 65535, np.uint16)
            svg.flat[:n] = src[order]
            etg.flat[:n] = typ[order]
            dvg.flat[:n] = dst[order]
            svs.append(svg); ets.append(etg); dvs.append(dvg)
            degts.append(cnt.astype(np.uint8).reshape(NT, P).T.copy())
            degcs.append(cnt.astype(np.uint8).reshape(NCHUNK, CW))
        wpna_l = pna_w[l].copy().reshape(12, D, D)
        for a in range(4):
            wpna_l[a * 3 + 1] /= dmean
            wpna_l[a * 3 + 2] *= dmean
        wpna_in = np.concatenate(
            [wpna_l.reshape(12 * D, D), pna_b[l][None, :]], axis=0)
        lay_in = {
            "sv": np.concatenate(svs, 0), "et": np.concatenate(ets, 0),
            "dv": np.concatenate(dvs, 0),
            "degt": np.concatenate(degts, 0), "degc": np.concatenate(degcs, 0),
            "wrel": relw[l],
            "wpna": wpna_in,
            "lh": lh_in, "w1": w1_in, "w2": w2_in, "b1": b1_in,
            "hb": hb_in, "b2": b2_in,
            "utab": utab_r, "hid": hid_r, "score": score_r, "zc": zc_r,
        }
        _t2 = _time.time()
        lay_bufs = _dev_zeros(layer_info)
        st = _call(layer_info, lay_in, lay_bufs)
        st["score_o"].block_until_ready()
        _t3 = _time.time()
        hid_r, utab_r, score_r = st["hid_o"], st["utab_o"], st["score_o"]
        score_h = np.asarray(score_r).reshape(NCORES_USED, NPAD)[:, :N].copy()
        if prof:
            print(f"[prof] L{l}: host_select={_t2-_t1:.3f}s call={_t3-_t2:.3f}s fetch={_time.time()-_t3:.3f}s", flush=True)

    out = np.zeros((B, T), _f32)
    for b in range(B):
        out[b] = score_h[b][t_index[b]]

    # exact host fallback for pathological batches (deg > LINKRUN)
    if fallback.any():
        for b in np.nonzero(fallback)[0]:
            out[b] = _host_batch(
                b, h_index, r_index, t_index, all_index, edge_src, edge_dst,
                edge_type, hidden_states, score_text_embs, rel_table,
                linear_w, linear_b, mlp_w1, mlp_b1, mlp_w2, mlp_b2,
                relw, pna_w, pna_b, dmean, sf)
    return out


def _mkhoff(hidx):
    a = np.full((P, 1), 1 << 20, _i32)
    a[0, 0] = hidx
    return a


def _host_batch(b, h_index, r_index, t_index, all_index, edge_src, edge_dst,
                edge_type, hidden_states, score_text_embs, rel_table,
                linear_w, linear_b, mlp_w1, mlp_b1, mlp_w2, mlp_b2,
                relw, pna_w, pna_b, dmean, sf):
    rel = rel_table[r_index[b]]
    hidden = np.zeros((N, D), _f32)
    hidden[all_index] = score_text_embs
    hidden[h_index[b]] = hidden_states[b]
    base = sf(np.zeros((1, D), _f32), rel)[0]
    score = np.full(N, base, _f32)
    score[h_index[b]] = sf(hidden_states[b][None], rel)[0]
    for l in range(L):
        nidx = np.argsort(-score, kind="stable")[:K]
        sel = np.zeros(N, bool)
        sel[nidx] = True
        escore = np.where(sel[edge_src], score[edge_dst], -np.inf).astype(_f32)
        eidx = np.argsort(-escore, kind="stable")[:ESEL]
        ev = escore[eidx]
        valid = np.isfinite(ev)
        s, d2, et = edge_src[eidx], edge_dst[eidx], edge_type[eidx]
        gate = _sigmoid(score)
        sv, dv, etv = s[valid], d2[valid], et[valid]
        msg = ((gate[sv, None] * hidden[sv]) * relw[l][etv]).astype(_f32)
        order = np.argsort(dv, kind="stable")
        ds = dv[order]
        ms = msg[order]
        uniq, starts = np.unique(ds, return_index=True)
        sm = np.zeros((N, D), _f32); sq = np.zeros((N, D), _f32)
        mx = np.zeros((N, D), _f32); mn = np.zeros((N, D), _f32)
        if len(uniq):
            sm[uniq] = np.add.reduceat(ms, starts, axis=0)
            sq[uniq] = np.add.reduceat((ms * ms).astype(_f32), starts, axis=0)
            mx[uniq] = np.maximum.reduceat(ms, starts, axis=0)
            mn[uniq] = np.minimum.reduceat(ms, starts, axis=0)
        deg = np.bincount(dv, minlength=N).astype(_f32)
        has = deg > 0.0
        degc = np.maximum(deg, 1.0)
        mean = (sm / degc[:, None]).astype(_f32)
        var = (sq / degc[:, None] - mean * mean).astype(_f32)
        std = np.where(has[:, None],
                       np.sqrt(np.maximum(var, 0.0) + _f32(1e-6), dtype=_f32),
                       0.0).astype(_f32)
        mx = np.where(has[:, None], mx, 0.0).astype(_f32)
        mn = np.where(has[:, None], mn, 0.0).astype(_f32)
        logd = np.log(deg + 1.0, dtype=_f32)
        ampv = (logd / dmean).astype(_f32)
        attv = np.where(has, dmean / np.maximum(logd, _f32(1e-6)), 0.0).astype(_f32)
        one = np.ones_like(ampv)
        feats = np.concatenate(
            [(a * sc[:, None]).astype(_f32)
             for a in (mean, mx, mn, std) for sc in (one, ampv, attv)], -1)
        outp = (feats @ pna_w[l] + pna_b[l]).astype(_f32)
        hidden = np.where(has[:, None], hidden + outp, hidden).astype(_f32)
        news = sf(hidden, rel)
        score = np.where(deg > 0.0, news, score).astype(_f32)
    return score[t_index[b]]


# revision 5
# speedup vs baseline: 1.0335x; 1.0335x over previous
"""ConditionedPNA on trn2: device-resident message passing + dense PNA.

4-core mesh, one query batch per core (data-parallel over B per the hint).
Device-resident per batch: hid [NPAD,64], utab [NPAD,64] (= sigmoid(score)*
hidden gather table), score [NPAD,1], zc [NPAD,256] (table init pattern).
Per layer: host selects top-K nodes / top-ESEL edges (exact jax tie
semantics) on compact arrays, ships dst-sorted edges as uint16 [128,1280]
slot grids; device gathers u/relw rows, forms msg|msg^2, segment-reduces
into smq [NPAD,256] (sum|sq|max|min) via gather-modify-scatter rounds
(slot column c = dst-sorted positions {p*1280+c}: same-dst runs are
contiguous and short, so every column and every 8-column stride-160 link
has unique dst -> indirect DMA semantics exact), then does the dense PNA
update + score MLP on-device. Only the new score ships back (200KB/batch).
"""
import os
import sys

sys.path.insert(0, "/opt/trn_rl_repo")

import numpy as np

# ---------------- problem constants ----------------
B, N, E, D, R2, T, M, L = 4, 50000, 1600000, 64, 1000, 32, 10000, 3
K = 5000
ESEL = 160000
P = 128
NT = 392                      # node tiles
NPAD = NT * P                 # 50176
NCHUNK = 98                   # 512-node chunks
CW = 512
SLOTC = 1280                  # slot columns
NSLOT = P * SLOTC             # 163840
LINKS = 160                   # GMS links (8 cols each, stride LINKS)
MAXRUN = 1280                 # layout-safe max in-degree for column unique
LINKRUN = 160                 # max run so stride-160 link stays unique
MT = 79                       # text tiles
MPAD = MT * P                 # 10112
NCORES_USED = 4

_f32 = np.float32
_i32 = np.int32

_STATE = None


# ================= device program builders =================
def _prog_init(nc):
    import concourse.tile as tile
    import concourse.bass as bass
    from concourse import mybir
    from concourse.masks import make_identity
    dt = mybir.dt
    AF = mybir.ActivationFunctionType

    t_texts = nc.dram_tensor("texts", [MPAD, D], dt.float32, kind="ExternalInput")
    t_alloff = nc.dram_tensor("alloff", [P, MT], dt.int32, kind="ExternalInput")
    t_hemb = nc.dram_tensor("hemb", [P, D], dt.float32, kind="ExternalInput")
    t_hoff = nc.dram_tensor("hoff", [P, 1], dt.int32, kind="ExternalInput")
    t_hsc = nc.dram_tensor("hsc", [P, 1], dt.float32, kind="ExternalInput")
    t_sc0 = nc.dram_tensor("sc0", [NPAD, 1], dt.float32, kind="ExternalInput")

    t_hid = nc.dram_tensor("hid", [NPAD, D], dt.float32, kind="ExternalOutput")
    t_utab = nc.dram_tensor("utab", [NPAD, D], dt.float32, kind="ExternalOutput")
    t_score = nc.dram_tensor("score", [NPAD, 1], dt.float32, kind="ExternalOutput")
    t_zc = nc.dram_tensor("zc", [NPAD, 256], dt.float32, kind="ExternalOutput")

    with tile.TileContext(nc) as tc:
        with (tc.tile_pool(name="big", bufs=1) as big,
              tc.tile_pool(name="wk", bufs=3) as wk,
              tc.tile_pool(name="ps", bufs=4, space="PSUM") as ps):
            ident = big.tile([P, P], dt.float32)
            make_identity(nc, ident[:])

            # zc pattern + zero hidden
            zpat = big.tile([P, 256], dt.float32)
            nc.vector.memset(zpat[:, 0:128], 0.0)
            nc.vector.memset(zpat[:, 128:192], -1e38)
            nc.vector.memset(zpat[:, 192:256], 1e38)
            for t in range(NT):
                nc.sync.dma_start(t_zc[t * P:(t + 1) * P, :], zpat[:])
                nc.sync.dma_start(t_hid[t * P:(t + 1) * P, :], zpat[:, 0:D])

            # scatter text embeddings (dedup'd, last-wins on host), then head
            txt = big.tile([P, MT * D], dt.float32)
            nc.sync.dma_start(
                txt[:].rearrange("p (c d) -> p c d", c=MT),
                t_texts[:].rearrange("(c p) d -> p c d", p=P))
            aoff = big.tile([P, MT], dt.int32)
            nc.sync.dma_start(aoff[:], t_alloff[:])
            for c in range(MT):
                nc.gpsimd.indirect_dma_start(
                    out=t_hid[:], out_offset=bass.IndirectOffsetOnAxis(
                        ap=aoff[:, c:c + 1], axis=0),
                    in_=txt[:, c * D:(c + 1) * D], in_offset=None,
                    bounds_check=NPAD - 1, oob_is_err=False)
            hemb = wk.tile([P, D], dt.float32, tag="hemb")
            nc.sync.dma_start(hemb[:], t_hemb[:])
            hoff = big.tile([P, 1], dt.int32)
            nc.sync.dma_start(hoff[:], t_hoff[:])
            nc.gpsimd.indirect_dma_start(
                out=t_hid[:], out_offset=bass.IndirectOffsetOnAxis(
                    ap=hoff[:, 0:1], axis=0),
                in_=hemb[:], in_offset=None,
                bounds_check=NPAD - 1, oob_is_err=False)

            # score0 (host grid) + head score scatter
            nc.sync.dma_start(t_score[:], t_sc0[:])
            hsc = wk.tile([P, 1], dt.float32, tag="hsc")
            nc.sync.dma_start(hsc[:], t_hsc[:])
            nc.gpsimd.indirect_dma_start(
                out=t_score[:], out_offset=bass.IndirectOffsetOnAxis(
                    ap=hoff[:, 0:1], axis=0),
                in_=hsc[:], in_offset=None,
                bounds_check=NPAD - 1, oob_is_err=False)

            # gate tile-major from score (chunk-major load + transposes)
            sC = big.tile([NCHUNK, CW], dt.float32)
            nc.sync.dma_start(
                sC[:], t_score[:].rearrange("(k w) o -> k (w o)", k=NCHUNK))
            gC = big.tile([NCHUNK, CW], dt.float32)
            nc.scalar.activation(gC[:], sC[:], AF.Sigmoid)
            gM = big.tile([P, NT], dt.float32)
            for q in range(4):
                pt = ps.tile([P, NCHUNK], dt.float32, tag="tg")
                nc.tensor.transpose(pt[:], gC[:, q * P:(q + 1) * P],
                                    ident[0:NCHUNK, 0:NCHUNK])
                nc.vector.tensor_copy(gM[:, q::4], pt[:])

            # utab = gate * hidden (node-major tiles)
            for t in range(NT):
                h0 = wk.tile([P, D], dt.float32, tag="h0")
                nc.sync.dma_start(h0[:], t_hid[t * P:(t + 1) * P, :])
                u0 = wk.tile([P, D], dt.float32, tag="u0")
                nc.vector.tensor_scalar_mul(u0[:], h0[:], gM[:, t:t + 1])
                nc.sync.dma_start(t_utab[t * P:(t + 1) * P, :], u0[:])


def _prog_layer(nc):
    import concourse.tile as tile
    import concourse.bass as bass
    from concourse import mybir
    from concourse.masks import make_identity
    dt = mybir.dt
    AOp = mybir.AluOpType
    AF = mybir.ActivationFunctionType

    t_sv = nc.dram_tensor("sv", [P, SLOTC], dt.uint16, kind="ExternalInput")
    t_et = nc.dram_tensor("et", [P, SLOTC], dt.uint16, kind="ExternalInput")
    t_dv = nc.dram_tensor("dv", [P, SLOTC], dt.uint16, kind="ExternalInput")
    t_degt = nc.dram_tensor("degt", [P, NT], dt.uint8, kind="ExternalInput")
    t_degc = nc.dram_tensor("degc", [NCHUNK, CW], dt.uint8, kind="ExternalInput")
    t_wrel = nc.dram_tensor("wrel", [R2, D], dt.float32, kind="ExternalInput")
    t_wpna = nc.dram_tensor("wpna", [12 * D + 1, D], dt.float32, kind="ExternalInput")
    t_lh = nc.dram_tensor("lh", [D, D], dt.float32, kind="ExternalInput")
    t_w1 = nc.dram_tensor("w1", [D, 2 * D], dt.float32, kind="ExternalInput")
    t_w2 = nc.dram_tensor("w2", [2 * D, 1], dt.float32, kind="ExternalInput")
    t_b1 = nc.dram_tensor("b1", [2 * D, 1], dt.float32, kind="ExternalInput")
    t_hb = nc.dram_tensor("hb", [D, 1], dt.float32, kind="ExternalInput")
    t_b2 = nc.dram_tensor("b2", [1, 1], dt.float32, kind="ExternalInput")
    t_utab = nc.dram_tensor("utab", [NPAD, D], dt.float32, kind="ExternalInput")
    t_hid = nc.dram_tensor("hid", [NPAD, D], dt.float32, kind="ExternalInput")
    t_score = nc.dram_tensor("score", [NPAD, 1], dt.float32, kind="ExternalInput")
    t_zc = nc.dram_tensor("zc", [NPAD, 256], dt.float32, kind="ExternalInput")

    t_hid_o = nc.dram_tensor("hid_o", [NPAD, D], dt.float32, kind="ExternalOutput")
    t_utab_o = nc.dram_tensor("utab_o", [NPAD, D], dt.float32, kind="ExternalOutput")
    t_score_o = nc.dram_tensor("score_o", [NPAD, 1], dt.float32, kind="ExternalOutput")

    t_smq = nc.dram_tensor("smq", [NPAD, 256], dt.float32, kind="Internal")
    t_sst = nc.dram_tensor("sst", [NPAD, 1], dt.float32, kind="Internal")

    with tile.TileContext(nc) as tc:
        with (tc.tile_pool(name="big", bufs=1) as big,
              tc.tile_pool(name="wk", bufs=4) as wk,
              tc.tile_pool(name="gat", bufs=8) as gat,
              tc.tile_pool(name="psT", bufs=2, space="PSUM") as psT,
              tc.tile_pool(name="psG", bufs=1, space="PSUM") as psG,
              tc.tile_pool(name="psS", bufs=1, space="PSUM") as psS):
            ident = big.tile([P, P], dt.float32)
            make_identity(nc, ident[:])
            ones128 = big.tile([1, P], dt.float32)
            nc.vector.memset(ones128[:], 1.0)

            # reset smq
            nc.sync.dma_start(t_smq[:], t_zc[:])

            # slot grids -> int32
            svi = big.tile([P, SLOTC], dt.int32)
            eti = big.tile([P, SLOTC], dt.int32)
            dvi = big.tile([P, SLOTC], dt.int32)
            for (src, dst) in ((t_sv, svi), (t_et, eti), (t_dv, dvi)):
                u16 = wk.tile([P, SLOTC], dt.uint16, tag="u16")
                nc.sync.dma_start(u16[:], src[:])
                nc.vector.tensor_copy(dst[:], u16[:])

            # ---- scatter phase: LINKS links x 8 columns (stride LINKS)
            for ln in range(LINKS):
                cols = [ln + LINKS * s for s in range(8)]
                pairs = []
                for c in cols:
                    gu = gat.tile([P, D], dt.float32, tag="gu")
                    nc.gpsimd.indirect_dma_start(
                        out=gu[:], out_offset=None, in_=t_utab[:],
                        in_offset=bass.IndirectOffsetOnAxis(
                            ap=svi[:, c:c + 1], axis=0),
                        bounds_check=NPAD - 1, oob_is_err=False)
                    gw = gat.tile([P, D], dt.float32, tag="gw")
                    nc.gpsimd.indirect_dma_start(
                        out=gw[:], out_offset=None, in_=t_wrel[:],
                        in_offset=bass.IndirectOffsetOnAxis(
                            ap=eti[:, c:c + 1], axis=0),
                        bounds_check=R2 - 1, oob_is_err=False)
                    pair = gat.tile([P, 2 * D], dt.float32, tag="pair")
                    nc.vector.tensor_mul(pair[:, 0:D], gu[:], gw[:])
                    nc.scalar.square(pair[:, D:2 * D], pair[:, 0:D])
                    pairs.append(pair)
                curs = []
                for c in cols:
                    cur = gat.tile([P, 256], dt.float32, tag="cur")
                    nc.gpsimd.indirect_dma_start(
                        out=cur[:], out_offset=None, in_=t_smq[:],
                        in_offset=bass.IndirectOffsetOnAxis(
                            ap=dvi[:, c:c + 1], axis=0),
                        bounds_check=NPAD - 1, oob_is_err=False)
                    curs.append(cur)
                for i, c in enumerate(cols):
                    pair, cur = pairs[i], curs[i]
                    upd = gat.tile([P, 256], dt.float32, tag="upd")
                    nc.vector.tensor_add(upd[:, 0:128], cur[:, 0:128], pair[:])
                    nc.vector.tensor_tensor(upd[:, 128:192], cur[:, 128:192],
                                            pair[:, 0:D], op=AOp.max)
                    nc.vector.tensor_tensor(upd[:, 192:256], cur[:, 192:256],
                                            pair[:, 0:D], op=AOp.min)
                    nc.gpsimd.indirect_dma_start(
                        out=t_smq[:], out_offset=bass.IndirectOffsetOnAxis(
                            ap=dvi[:, c:c + 1], axis=0),
                        in_=upd[:], in_offset=None,
                        bounds_check=NPAD - 1, oob_is_err=False)

            # ---- per-node scalars (tile-major)
            degt8 = big.tile([P, NT], dt.uint8)
            nc.sync.dma_start(degt8[:], t_degt[:])
            degf = big.tile([P, NT], dt.float32)
            nc.vector.tensor_copy(degf[:], degt8[:])
            hasf = big.tile([P, NT], dt.float32)
            nc.vector.tensor_scalar(hasf[:], degf[:], 0.0, None, op0=AOp.is_gt)
            hasu8 = big.tile([P, NT], dt.uint8)
            nc.vector.tensor_scalar(hasu8[:], degf[:], 0.0, None, op0=AOp.is_gt)
            dgc = wk.tile([P, NT], dt.float32, tag="dgc")
            nc.vector.tensor_scalar_max(dgc[:], degf[:], 1.0)
            recipd = big.tile([P, NT], dt.float32)
            nc.vector.reciprocal(recipd[:], dgc[:])
            ampM = big.tile([P, NT], dt.float32)
            nc.scalar.activation(ampM[:], degf[:], AF.Ln, bias=1.0)
            logc = wk.tile([P, NT], dt.float32, tag="logc")
            nc.vector.tensor_scalar_max(logc[:], ampM[:], 1e-6)
            attM = big.tile([P, NT], dt.float32)
            nc.vector.reciprocal(attM[:], logc[:])
            nc.vector.tensor_mul(attM[:], attM[:], hasf[:])

            # ---- weights
            wpna = big.tile([D, 12 * D], dt.float32)
            nc.sync.dma_start(
                wpna[:].rearrange("k (b d) -> k b d", b=12),
                t_wpna[0:12 * D, :].rearrange("(b k) d -> k b d", k=D))
            pnarow = big.tile([1, D], dt.float32)
            nc.sync.dma_start(pnarow[:], t_wpna[12 * D:12 * D + 1, :])
            pb_ps = psT.tile([P, P], dt.float32, tag="t")
            nc.tensor.matmul(pb_ps[:, 0:D], ones128[:], pnarow[:], start=True, stop=True)
            pnabB = big.tile([P, D], dt.float32)
            nc.vector.tensor_copy(pnabB[:], pb_ps[:, 0:D])
            lh = big.tile([D, D], dt.float32)
            nc.sync.dma_start(lh[:], t_lh[:])
            w1 = big.tile([D, 2 * D], dt.float32)
            nc.sync.dma_start(w1[:], t_w1[:])
            w2 = big.tile([2 * D, 1], dt.float32)
            nc.sync.dma_start(w2[:], t_w2[:])
            b1 = big.tile([2 * D, 1], dt.float32)
            nc.sync.dma_start(b1[:], t_b1[:])
            hb = big.tile([D, 1], dt.float32)
            nc.sync.dma_start(hb[:], t_hb[:])
            b2 = big.tile([1, 1], dt.float32)
            nc.sync.dma_start(b2[:], t_b2[:])

            # ---- PNA + score_fn per 512-node chunk
            for k in range(NCHUNK):
                a_mean = wk.tile([D, CW], dt.float32, tag="a_mean")
                a_std = wk.tile([D, CW], dt.float32, tag="a_std")
                a_mx = wk.tile([D, CW], dt.float32, tag="a_mx")
                a_mn = wk.tile([D, CW], dt.float32, tag="a_mn")
                hTc = wk.tile([D, CW], dt.float32, tag="hTc")
                for t4 in range(4):
                    t = 4 * k + t4
                    smt = wk.tile([P, 256], dt.float32, tag="smt")
                    nc.sync.dma_start(smt[:], t_smq[t * P:(t + 1) * P, :])
                    rd = recipd[:, t:t + 1]
                    hs = hasf[:, t:t + 1]
                    agg = wk.tile([P, 256], dt.float32, tag="agg")
                    nc.vector.tensor_scalar_mul(agg[:, 0:D], smt[:, 0:D], rd)
                    ex2 = wk.tile([P, D], dt.float32, tag="ex2")
                    nc.vector.tensor_scalar_mul(ex2[:], smt[:, D:2 * D], rd)
                    msq = wk.tile([P, D], dt.float32, tag="msq")
                    nc.scalar.square(msq[:], agg[:, 0:D])
                    nc.vector.tensor_sub(ex2[:], ex2[:], msq[:])
                    nc.vector.tensor_scalar(ex2[:], ex2[:], 0.0, 1e-6,
                                            op0=AOp.max, op1=AOp.add)
                    nc.scalar.sqrt(ex2[:], ex2[:])
                    nc.vector.tensor_scalar_mul(agg[:, D:2 * D], ex2[:], hs)
                    nc.vector.tensor_scalar_mul(agg[:, 2 * D:3 * D],
                                                smt[:, 2 * D:3 * D], hs)
                    nc.vector.tensor_scalar_mul(agg[:, 3 * D:4 * D],
                                                smt[:, 3 * D:4 * D], hs)
                    pt = psT.tile([P, P], dt.float32, tag="t")
                    nc.tensor.transpose(pt[:], agg[:, 0:128], ident[:])
                    sl = slice(t4 * P, (t4 + 1) * P)
                    nc.vector.tensor_copy(a_mean[:, sl], pt[0:D, :])
                    nc.vector.tensor_copy(a_std[:, sl], pt[D:2 * D, :])
                    pt2 = psT.tile([P, P], dt.float32, tag="t")
                    nc.tensor.transpose(pt2[:], agg[:, 128:256], ident[:])
                    nc.vector.tensor_copy(a_mx[:, sl], pt2[0:D, :])
                    nc.vector.tensor_copy(a_mn[:, sl], pt2[D:2 * D, :])

                aggs = (a_mean, a_mx, a_mn, a_std)
                for t4 in range(4):
                    t = 4 * k + t4
                    sl = slice(t4 * P, (t4 + 1) * P)
                    pgs = []
                    for s in range(3):
                        pg = psG.tile([P, D], dt.float32, tag=f"pg{s}")
                        for a in range(4):
                            bidx = a * 3 + s
                            nc.tensor.matmul(
                                pg[:], aggs[a][:, sl],
                                wpna[:, bidx * D:(bidx + 1) * D],
                                start=(a == 0), stop=(a == 3))
                        pgs.append(pg)
                    hload = wk.tile([P, D], dt.float32, tag="hload")
                    nc.sync.dma_start(hload[:], t_hid[t * P:(t + 1) * P, :])
                    o2 = wk.tile([P, D], dt.float32, tag="o2")
                    nc.vector.tensor_scalar_mul(o2[:], pgs[1][:], ampM[:, t:t + 1])
                    o3 = wk.tile([P, D], dt.float32, tag="o3")
                    nc.vector.tensor_scalar_mul(o3[:], pgs[2][:], attM[:, t:t + 1])
                    out = wk.tile([P, D], dt.float32, tag="outc")
                    nc.vector.tensor_add(out[:], pgs[0][:], o2[:])
                    nc.vector.tensor_add(out[:], out[:], o3[:])
                    nc.vector.tensor_add(out[:], out[:], pnabB[:])
                    hnew = wk.tile([P, D], dt.float32, tag="hnew")
                    nc.vector.tensor_add(hnew[:], hload[:], out[:])
                    hsel = wk.tile([P, D], dt.float32, tag="hsel")
                    nc.vector.select(
                        hsel[:], hasu8[:, t:t + 1].to_broadcast([P, D]),
                        hnew[:], hload[:])
                    nc.sync.dma_start(t_hid_o[t * P:(t + 1) * P, :], hsel[:])
                    pth = psT.tile([P, P], dt.float32, tag="t")
                    nc.tensor.transpose(pth[0:D, :], hsel[:], ident[:])
                    nc.vector.tensor_copy(hTc[:, sl], pth[0:D, :])

                # score_fn on this chunk
                ph = psS.tile([D, CW], dt.float32, tag="ph")
                nc.tensor.matmul(ph[:], lh[:], hTc[:], start=True, stop=True)
                xt = wk.tile([D, CW], dt.float32, tag="xt")
                nc.vector.tensor_scalar_add(xt[:], ph[:], hb[:, 0:1])
                nc.vector.tensor_mul(xt[:], xt[:], hTc[:])
                p1 = psS.tile([2 * D, CW], dt.float32, tag="p1")
                nc.tensor.matmul(p1[:], w1[:], xt[:], start=True, stop=True)
                h1 = wk.tile([2 * D, CW], dt.float32, tag="h1")
                nc.scalar.activation(h1[:], p1[:], AF.Relu, bias=b1[:, 0:1])
                p2s = psS.tile([1, CW], dt.float32, tag="p2s")
                nc.tensor.matmul(p2s[:], w2[:], h1[:], start=True, stop=True)
                srow = wk.tile([1, CW], dt.float32, tag="srow")
                nc.vector.tensor_scalar_add(srow[:], p2s[:], b2[0:1, 0:1])
                nc.sync.dma_start(
                    t_sst[k * CW:(k + 1) * CW, :].rearrange("n o -> o n"),
                    srow[:])

            # ---- score select + gate + utab
            sC = big.tile([NCHUNK, CW], dt.float32)
            nc.sync.dma_start(
                sC[:], t_sst[:].rearrange("(k w) o -> k (w o)", k=NCHUNK))
            soldC = big.tile([NCHUNK, CW], dt.float32)
            nc.sync.dma_start(
                soldC[:], t_score[:].rearrange("(k w) o -> k (w o)", k=NCHUNK))
            dc8 = wk.tile([NCHUNK, CW], dt.uint8, tag="dc8")
            nc.sync.dma_start(dc8[:], t_degc[:])
            dcf = wk.tile([NCHUNK, CW], dt.float32, tag="dcf")
            nc.vector.tensor_copy(dcf[:], dc8[:])
            hasC = big.tile([NCHUNK, CW], dt.uint8)
            nc.vector.tensor_scalar(hasC[:], dcf[:], 0.0, None, op0=AOp.is_gt)
            snewC = big.tile([NCHUNK, CW], dt.float32)
            nc.vector.select(snewC[:], hasC[:], sC[:], soldC[:])
            nc.sync.dma_start(
                t_score_o[:].rearrange("(k w) o -> k (w o)", k=NCHUNK), snewC[:])
            gC = big.tile([NCHUNK, CW], dt.float32)
            nc.scalar.activation(gC[:], snewC[:], AF.Sigmoid)
            gM = big.tile([P, NT], dt.float32)
            for q in range(4):
                ptg = psT.tile([P, P], dt.float32, tag="t")
                nc.tensor.transpose(ptg[:, 0:NCHUNK], gC[:, q * P:(q + 1) * P],
                                    ident[0:NCHUNK, 0:NCHUNK])
                nc.vector.tensor_copy(gM[:, q::4], ptg[:, 0:NCHUNK])
            for t in range(NT):
                hload = wk.tile([P, D], dt.float32, tag="hl2")
                nc.sync.dma_start(hload[:], t_hid_o[t * P:(t + 1) * P, :])
                u0 = wk.tile([P, D], dt.float32, tag="u0")
                nc.vector.tensor_scalar_mul(u0[:], hload[:], gM[:, t:t + 1])
                nc.sync.dma_start(t_utab_o[t * P:(t + 1) * P, :], u0[:])


# ================= runner plumbing =================
def _make_runner(build_fn, donate_in=(), replicated=()):
    import concourse.bacc as bacc
    from concourse.bass2jax import (
        install_neuronx_cc_hook, _bass_exec_p, partition_id_tensor,
    )
    import jax
    from jax.sharding import Mesh, PartitionSpec
    from jax.experimental.shard_map import shard_map
    from concourse import mybir as mb

    nc = bacc.Bacc(target_bir_lowering=False)
    build_fn(nc)
    nc.finalize()
    install_neuronx_cc_hook()

    partition_name = nc.partition_id_tensor.name if nc.partition_id_tensor else None
    in_names, out_names, out_avals = [], [], []
    for alloc in nc.m.functions[0].allocations:
        if not isinstance(alloc, mb.MemoryLocationSet):
            continue
        name = alloc.memorylocations[0].name
        if alloc.kind == "ExternalInput":
            if name != partition_name:
                in_names.append(name)
        elif alloc.kind == "ExternalOutput":
            out_names.append(name)
            out_avals.append(jax.core.ShapedArray(
                tuple(alloc.tensor_shape), mb.dt.np(alloc.dtype)))
    all_names = list(in_names) + list(out_names)
    if partition_name is not None:
        all_names.append(partition_name)

    def _body(*args):
        operands = list(args)
        if partition_name is not None:
            operands.append(partition_id_tensor())
        return tuple(_bass_exec_p.bind(
            *operands, out_avals=tuple(out_avals), in_names=tuple(all_names),
            out_names=tuple(out_names), lowering_input_output_aliases=(),
            sim_require_finite=False, sim_require_nnan=False, nc=nc))

    devices = jax.devices()[:NCORES_USED]
    mesh = Mesh(np.asarray(devices), ("core",))
    nin = len(in_names)
    nout = len(out_avals)
    donate = tuple(in_names.index(n) for n in donate_in) + tuple(
        range(nin, nin + nout))
    in_specs = tuple(
        PartitionSpec() if n in replicated else PartitionSpec("core")
        for n in in_names) + (PartitionSpec("core"),) * nout
    sharded = jax.jit(shard_map(_body, mesh=mesh,
                                in_specs=in_specs,
                                out_specs=(PartitionSpec("core"),) * nout,
                                check_rep=False),
                      donate_argnums=donate, keep_unused=True)
    return {"in_names": in_names, "out_names": out_names,
            "out_avals": out_avals, "sharded": sharded, "mesh": mesh}


def _dev_zeros_factory():
    import jax
    import jax.numpy as jnp
    from jax.sharding import NamedSharding, PartitionSpec
    cache = {}

    def make(info):
        key = id(info)
        if key not in cache:
            mesh = info["mesh"]
            sh = NamedSharding(mesh, PartitionSpec("core"))
            fns = []
            for av in info["out_avals"]:
                shape = (NCORES_USED * av.shape[0],) + tuple(av.shape[1:])
                fns.append(jax.jit(
                    lambda shape=shape, dt=av.dtype: jnp.zeros(shape, dt),
                    out_shardings=sh))
            cache[key] = fns
        return [f() for f in cache[key]]
    return make


_dev_zeros = None


def _get_programs():
    global _STATE, _dev_zeros
    if _STATE is None:
        init_info = _make_runner(_prog_init)
        layer_info = _make_runner(_prog_layer,
                                  donate_in=("utab", "hid", "score"))
        _dev_zeros = _dev_zeros_factory()
        _STATE = (init_info, layer_info)
    return _STATE


def _call(info, arrays_by_name, out_bufs):
    args = [arrays_by_name[n] for n in info["in_names"]]
    outs = info["sharded"](*args, *out_bufs)
    return dict(zip(info["out_names"], outs))


# ================= host-side exact selection =================
def _sigmoid(x):
    x = x.astype(_f32)
    out = np.empty_like(x)
    pos = x >= 0
    out[pos] = (1.0 / (1.0 + np.exp(-x[pos]))).astype(_f32)
    ex = np.exp(x[~pos]).astype(_f32)
    out[~pos] = ex / (1.0 + ex)
    return out.astype(_f32)


def _score_fn(hidden, rel, linear_w, linear_b, mlp_w1, mlp_b1, mlp_w2, mlp_b2):
    heur = hidden @ linear_w[:D] + rel @ linear_w[D:] + linear_b
    x = hidden * heur
    h1 = np.maximum(x @ mlp_w1 + mlp_b1, 0.0)
    return (h1 @ mlp_w2 + mlp_b2).astype(_f32)[:, 0]


def _topk_sel(score, k):
    """Exact lax.top_k set: k largest, ties -> lowest index."""
    v = np.partition(score, N - k)[N - k]
    gt = score > v
    g = int(gt.sum())
    sel = gt
    if g < k:
        eq_idx = np.nonzero(score == v)[0][:k - g]
        sel = gt.copy()
        sel[eq_idx] = True
    return sel


def _select_edges(score, sel, csr):
    """Exact top-ESEL edge set (ties by lowest original edge index).
    Returns (src, dst, typ) arrays of the selected valid edges."""
    indptr, sdst, styp, sorder, sel_src = csr
    sel_nodes = np.nonzero(sel)[0]
    st = indptr[sel_nodes]
    en = indptr[sel_nodes + 1]
    lens = en - st
    tot = int(lens.sum())
    if tot == 0:
        z = np.zeros(0, _i32)
        return z, z, z
    base = np.repeat(st - np.concatenate([[0], np.cumsum(lens)[:-1]]), lens)
    cand = base + np.arange(tot, dtype=np.int64)
    esc = score[sdst[cand]]
    if tot <= ESEL:
        keep = np.ones(tot, bool)
    else:
        v = np.partition(esc, tot - ESEL)[tot - ESEL]
        keep = esc > v
        g = int(keep.sum())
        need = ESEL - g
        if need > 0:
            eqm = esc == v
            eq_pos = np.nonzero(eqm)[0]
            eor = sorder[cand[eq_pos]]
            if len(eq_pos) > need:
                kth = np.partition(eor, need - 1)[need - 1]
                sub = eor <= kth
                keep[eq_pos[sub]] = True
            else:
                keep[eq_pos] = True
    candk = cand[keep]
    src = np.repeat(sel_nodes, lens)[keep].astype(_i32)
    dst = sdst[candk]
    typ = styp[candk]
    return src, dst, typ


# ================= main =================
def kernel(h_index, r_index, t_index, all_index, edge_src, edge_dst, edge_type,
           hidden_states, score_text_embs, rel_table, linear_w, linear_b,
           mlp_w1, mlp_b1, mlp_w2, mlp_b2, relw, pna_w, pna_b):
    h_index = np.asarray(h_index)
    r_index = np.asarray(r_index)
    t_index = np.asarray(t_index)
    all_index = np.asarray(all_index)
    edge_src = np.asarray(edge_src).astype(np.int64)
    edge_dst = np.asarray(edge_dst).astype(np.int64)
    edge_type = np.asarray(edge_type).astype(np.int64)
    hidden_states = np.asarray(hidden_states, _f32)
    score_text_embs = np.asarray(score_text_embs, _f32)
    rel_table = np.asarray(rel_table, _f32)
    linear_w = np.asarray(linear_w, _f32)
    linear_b = np.asarray(linear_b, _f32)
    mlp_w1 = np.asarray(mlp_w1, _f32)
    mlp_b1 = np.asarray(mlp_b1, _f32)
    mlp_w2 = np.asarray(mlp_w2, _f32)
    mlp_b2 = np.asarray(mlp_b2, _f32)
    relw = np.asarray(relw, _f32)
    pna_w = np.asarray(pna_w, _f32)
    pna_b = np.asarray(pna_b, _f32)

    deg_out_full = np.bincount(edge_src, minlength=N).astype(_f32)
    dmean = float(np.mean(np.log(deg_out_full + 1.0, dtype=_f32), dtype=_f32))

    # CSR by src (once)
    sorder = np.argsort(edge_src, kind="stable").astype(np.int64)
    ssrc = edge_src[sorder].astype(_i32)
    sdst = edge_dst[sorder].astype(_i32)
    styp = edge_type[sorder].astype(_i32)
    indptr = np.zeros(N + 1, np.int64)
    np.cumsum(np.bincount(ssrc, minlength=N), out=indptr[1:])
    csr = (indptr, sdst, styp, sorder.astype(np.int64), ssrc)

    sf = lambda h, r: _score_fn(h, r, linear_w, linear_b,
                                mlp_w1, mlp_b1, mlp_w2, mlp_b2)

    init_info, layer_info = _get_programs()

    # ---- init call inputs
    # dedup all_index keeping LAST occurrence (matches .at[].set order)
    rev_uniq, rev_first = np.unique(all_index[::-1], return_index=True)
    last_pos = len(all_index) - 1 - rev_first       # original positions
    mu = len(rev_uniq)
    texts = np.zeros((MPAD, D), _f32)
    alloff = np.full((P, MT), 1 << 20, _i32)
    # text slot (p, c) <-> texts row c*128+p
    tr = np.zeros(MPAD, np.int64)
    tr[:mu] = rev_uniq
    texts[:mu] = score_text_embs[last_pos]
    alloff.T.flat[:mu] = rev_uniq.astype(_i32)      # row-major over (c, p)

    base_sc = np.empty(B, _f32)
    head_sc = np.empty(B, _f32)
    for b in range(B):
        rel = rel_table[r_index[b]]
        base_sc[b] = sf(np.zeros((1, D), _f32), rel)[0]
        head_sc[b] = sf(hidden_states[b][None], rel)[0]

    def catb(fn):
        return np.concatenate([fn(b) for b in range(NCORES_USED)], axis=0)

    init_in = {
        "texts": catb(lambda b: texts),
        "alloff": catb(lambda b: alloff),
        "hemb": catb(lambda b: np.broadcast_to(hidden_states[b], (P, D)).copy()),
        "hoff": catb(lambda b: _mkhoff(int(h_index[b]))),
        "hsc": catb(lambda b: np.full((P, 1), head_sc[b], _f32)),
        "sc0": catb(lambda b: np.full((NPAD, 1), base_sc[b], _f32)),
    }
    prof = bool(os.environ.get("PNA_PROF"))
    import time as _time
    _t0 = _time.time()
    init_bufs = _dev_zeros(init_info)
    st = _call(init_info, init_in, init_bufs)
    hid_r, utab_r, score_r, zc_r = (st["hid"], st["utab"], st["score"], st["zc"])

    # host copy of score for selection
    score_h = np.asarray(score_r).reshape(NCORES_USED, NPAD)[:, :N].copy()
    if prof:
        print(f"[prof] init call+fetch: {_time.time()-_t0:.3f}s", flush=True)

    # per-layer static weights
    lh_in = catb(lambda b: linear_w[:D])
    w1_in = catb(lambda b: mlp_w1)
    w2_in = catb(lambda b: mlp_w2)
    b1_in = catb(lambda b: mlp_b1[:, None].astype(_f32))
    hb_in = catb(lambda b: (rel_table[r_index[b]] @ linear_w[D:]
                            + linear_b)[:, None].astype(_f32))
    b2_in = catb(lambda b: mlp_b2[:, None].astype(_f32))

    fallback = np.zeros(B, bool)
    for l in range(L):
        _t1 = _time.time()
        svs, ets, dvs, degts, degcs = [], [], [], [], []
        for b in range(B):
            sel = _topk_sel(score_h[b], K)
            src, dst, typ = _select_edges(score_h[b], sel, csr)
            cnt = np.bincount(dst, minlength=NPAD)
            if cnt.max() > LINKRUN:
                fallback[b] = True
                src = src[:0]; dst = dst[:0]; typ = typ[:0]
                cnt[:] = 0
            order = np.argsort(dst)
            n = len(order)
            svg = np.zeros((P, SLOTC), np.uint16)
            etg = np.zeros((P, SLOTC), np.uint16)
            dvg = np.full((P, SLOTC), 65535, np.uint16)
            svg.flat[:n] = src[order]
            etg.flat[:n] = typ[order]
            dvg.flat[:n] = dst[order]
            svs.append(svg); ets.append(etg); dvs.append(dvg)
            degts.append(cnt.astype(np.uint8).reshape(NT, P).T.copy())
            degcs.append(cnt.astype(np.uint8).reshape(NCHUNK, CW))
        wpna_l = pna_w[l].copy().reshape(12, D, D)
        for a in range(4):
            wpna_l[a * 3 + 1] /= dmean
            wpna_l[a * 3 + 2] *= dmean
        wpna_in = np.concatenate(
            [wpna_l.reshape(12 * D, D), pna_b[l][None, :]], axis=0)
        lay_in = {
            "sv": np.concatenate(svs, 0), "et": np.concatenate(ets, 0),
            "dv": np.concatenate(dvs, 0),
            "degt": np.concatenate(degts, 0), "degc": np.concatenate(degcs, 0),
            "wrel": catb(lambda b: relw[l]),
            "wpna": catb(lambda b: wpna_in),
            "lh": lh_in, "w1": w1_in, "w2": w2_in, "b1": b1_in,
            "hb": hb_in, "b2": b2_in,
            "utab": utab_r, "hid": hid_r, "score": score_r, "zc": zc_r,
        }
        _t2 = _time.time()
        lay_bufs = _dev_zeros(layer_info)
        st = _call(layer_info, lay_in, lay_bufs)
        st["score_o"].block_until_ready()
        _t3 = _time.time()
        hid_r, utab_r, score_r = st["hid_o"], st["utab_o"], st["score_o"]
        score_h = np.asarray(score_r).reshape(NCORES_USED, NPAD)[:, :N].copy()
        if prof:
            print(f"[prof] L{l}: host_select={_t2-_t1:.3f}s call={_t3-_t2:.3f}s fetch={_time.time()-_t3:.3f}s", flush=True)

    out = np.zeros((B, T), _f32)
    for b in range(B):
        out[b] = score_h[b][t_index[b]]

    # exact host fallback for pathological batches (deg > LINKRUN)
    if fallback.any():
        for b in np.nonzero(fallback)[0]:
            out[b] = _host_batch(
                b, h_index, r_index, t_index, all_index, edge_src, edge_dst,
                edge_type, hidden_states, score_text_embs, rel_table,
                linear_w, linear_b, mlp_w1, mlp_b1, mlp_w2, mlp_b2,
                relw, pna_w, pna_b, dmean, sf)
    return out


def _mkhoff(hidx):
    a = np.full((P, 1), 1 << 20, _i32)
    a[0, 0] = hidx
    return a


def _host_batch(b, h_index, r_index, t_index, all_index, edge_src, edge_dst,
                edge_type, hidden_states, score_text_embs, rel_table,
                linear_w, linear_b, mlp_w1, mlp_b1, mlp_w2, mlp_b2,
                relw, pna_w, pna_b, dmean, sf):
    rel = rel_table[r_index[b]]
    hidden = np.zeros((N, D), _f32)
    hidden[all_index] = score_text_embs
    hidden[h_index[b]] = hidden_states[b]
    base = sf(np.zeros((1, D), _f32), rel)[0]
    score = np.full(N, base, _f32)
    score[h_index[b]] = sf(hidden_states[b][None], rel)[0]
    for l in range(L):
        nidx = np.argsort(-score, kind="stable")[:K]
        sel = np.zeros(N, bool)
        sel[nidx] = True
        escore = np.where(sel[edge_src], score[edge_dst], -np.inf).astype(_f32)
        eidx = np.argsort(-escore, kind="stable")[:ESEL]
        ev = escore[eidx]
        valid = np.isfinite(ev)
        s, d2, et = edge_src[eidx], edge_dst[eidx], edge_type[eidx]
        gate = _sigmoid(score)
        sv, dv, etv = s[valid], d2[valid], et[valid]
        msg = ((gate[sv, None] * hidden[sv]) * relw[l][etv]).astype(_f32)
        order = np.argsort(dv, kind="stable")
        ds = dv[order]
        ms = msg[order]
        uniq, starts = np.unique(ds, return_index=True)
        sm = np.zeros((N, D), _f32); sq = np.zeros((N, D), _f32)
        mx = np.zeros((N, D), _f32); mn = np.zeros((N, D), _f32)
        if len(uniq):
            sm[uniq] = np.add.reduceat(ms, starts, axis=0)
            sq[uniq] = np.add.reduceat((ms * ms).astype(_f32), starts, axis=0)
            mx[uniq] = np.maximum.reduceat(ms, starts, axis=0)
            mn[uniq] = np.minimum.reduceat(ms, starts, axis=0)
        deg = np.bincount(dv, minlength=N).astype(_f32)
        has = deg > 0.0
        degc = np.maximum(deg, 1.0)
        mean = (sm / degc[:, None]).astype(_f32)
        var = (sq / degc[:, None] - mean * mean).astype(_f32)
        std = np.where(has[:, None],
                       np.sqrt(np.maximum(var, 0.0) + _f32(1e-6), dtype=_f32),
                       0.0).astype(_f32)
        mx = np.where(has[:, None], mx, 0.0).astype(_f32)
        mn = np.where(has[:, None], mn, 0.0).astype(_f32)
        logd = np.log(deg + 1.0, dtype=_f32)
        ampv = (logd / dmean).astype(_f32)
        attv = np.where(has, dmean / np.maximum(logd, _f32(1e-6)), 0.0).astype(_f32)
        one = np.ones_like(ampv)
        feats = np.concatenate(
            [(a * sc[:, None]).astype(_f32)
             for a in (mean, mx, mn, std) for sc in (one, ampv, attv)], -1)
        outp = (feats @ pna_w[l] + pna_b[l]).astype(_f32)
        hidden = np.where(has[:, None], hidden + outp, hidden).astype(_f32)
        news = sf(hidden, rel)
        score = np.where(deg > 0.0, news, score).astype(_f32)
    return score[t_index[b]]
